# revision 9
# baseline (speedup 1.0000x reference)
"""GQA attention (B=2, T=2048, D=4096, H=32, G=8, d=128) on 8 TRN2 NeuronCores.

Sharding: one KV group per core (4 Q heads + 1 K/V head). Each core:
  - projects q/k/v for its group in transposed ("d-major") layout,
  - causal attention with transposed score tiles S.T = K.T-chunk @ Q-tile,
    exp on ACT; the P@V matmul is flipped (stationary = P chunk, moving =
    V tile with a ones-column appended) so y comes out token-major AND the
    softmax row-sums accumulate for free in PSUM column 128,
  - normalization is then a per-partition reciprocal+scale on DVE,
  - y tiles are transposed back to d-major at the start of phase 3,
    pipelined with the partial out-projection against 512 columns of Wo.
Host sums the 8 bf16 partial outputs in f32 and adds bo.

All matmuls in bf16 with fp32 PSUM accumulation. Score tiles on the causal
diagonal are narrowed to the valid trapezoid (matmul N, exp width).
"""

import math
import sys

import numpy as np

sys.path.insert(0, "/opt/trn_rl_repo")

import ml_dtypes

BF16 = ml_dtypes.bfloat16

B, T, D = 2, 2048, 4096
H, G, d = 32, 8, 128
GROUP = H // G  # 4 heads per group/core
NT = B * T  # 4096 tokens
NC_ = 8  # cores

TOK = 512  # q-token tile (free dim of score matmuls, psum bank)
NTT = NT // TOK  # 8
DC = D // 128  # 32 contraction chunks
XC = 8  # Dc chunks per x DMA chunk (1MB each)
NXC = DC // XC  # 4 x-chunks per token tile
KT_PER_B = T // 128  # 16 k-tiles per batch

_program_cache = {}


def _build_program():
    import concourse.mybir as mybir
    import concourse.tile as tile
    from concourse import bacc
    from concourse.bass import ds, ts
    from concourse.masks import make_identity

    f32 = mybir.dt.float32
    bf16 = mybir.dt.bfloat16
    AF = mybir.ActivationFunctionType

    nc = bacc.Bacc()

    xt_d = nc.declare_dram_parameter("xt", [128, DC, NT], bf16, isOutput=False)
    wq_d = nc.declare_dram_parameter("wq", [128, DC * GROUP, 128], bf16, isOutput=False)
    wk_d = nc.declare_dram_parameter("wk", [128, DC, 128], bf16, isOutput=False)
    wv_d = nc.declare_dram_parameter("wv", [128, DC, 128], bf16, isOutput=False)
    wo_d = nc.declare_dram_parameter("wo", [128, GROUP * DC, 128], bf16, isOutput=False)
    bq_d = nc.declare_dram_parameter("bq", [128, GROUP], f32, isOutput=False)
    bk_d = nc.declare_dram_parameter("bk", [128, 1], f32, isOutput=False)
    bv_d = nc.declare_dram_parameter("bv", [128, 1], f32, isOutput=False)
    mask_d = nc.declare_dram_parameter("mask0", [128, 128], bf16, isOutput=False)
    out_d = nc.declare_dram_parameter("out", [128, DC, NT], bf16, isOutput=True)
    dbgy_d = nc.declare_dram_parameter(
        "dbgy", [128, B * GROUP * (T // 128), 128], bf16, isOutput=True
    )
    dbgl_d = nc.declare_dram_parameter(
        "dbgl", [128, B * GROUP * (T // 128)], f32, isOutput=True
    )

    with tile.TileContext(nc) as tc:
        with tc.tile_pool(name="persist", bufs=1) as persist:
            qT = persist.tile([128, GROUP, NT], bf16)  # [dq_row, head, tok]
            kT = persist.tile([128, NT], bf16)  # [d, tok]
            # v token-major tiles + ones column for free softmax row-sums
            vtm = persist.tile([128, NT // 128, 129], bf16)
            # y token-major: [tok%128, (b,h,qi,s), dv]
            yq_all = persist.tile([128, B * GROUP * (T // 128), 128], bf16)
            mask0 = persist.tile([128, 128], bf16)
            bq_s = persist.tile([128, GROUP], f32)
            bk_s = persist.tile([128, 1], f32)
            bv_s = persist.tile([128, 1], f32)
            ident = persist.tile([128, 128], bf16)
            l_stage = persist.tile([128, B * GROUP * (T // 128)], f32)

            # ---------------- Phase 1: q/k/v projections ----------------
            with (
                tc.tile_pool(name="wproj", bufs=1) as wpool,
                tc.tile_pool(name="xtp", bufs=8) as xpool,
                tc.tile_pool(name="vtstage", bufs=1) as vtp,
                tc.psum_pool(name="ps1", bufs=4) as ps1,
                tc.psum_pool(name="pstr", bufs=2) as pstr,
            ):
                wk_s = wpool.tile([128, DC, 128], bf16)
                wv_s = wpool.tile([128, DC, 128], bf16)
                wq_s = wpool.tile([128, GROUP * DC, 128], bf16)

                # critical-path-first DMA ordering: the very first matmul
                # needs only wk chunk 0 + x chunk (0,0).
                nc.sync.dma_start(out=bk_s[:], in_=bk_d[:])
                nc.sync.dma_start(out=wk_s[:, 0:XC, :], in_=wk_d[:, 0:XC, :])
                xch = {}
                for tt in range(2):  # prefetch two token tiles of x
                    for c in range(NXC):
                        xc = xpool.tile([128, XC, TOK], bf16, tag="xt")
                        xch[(tt, c)] = xc
                        nc.sync.dma_start(
                            out=xc[:], in_=xt_d[:, c * XC : (c + 1) * XC, ts(tt, TOK)]
                        )
                        if tt == 0 and c == 0:
                            for cc in range(1, NXC):
                                nc.sync.dma_start(
                                    out=wk_s[:, cc * XC : (cc + 1) * XC, :],
                                    in_=wk_d[:, cc * XC : (cc + 1) * XC, :],
                                )
                nc.sync.dma_start(out=bv_s[:], in_=bv_d[:])
                nc.sync.dma_start(out=bq_s[:], in_=bq_d[:])
                nc.sync.dma_start(out=wv_s[:], in_=wv_d[:])
                nc.sync.dma_start(out=mask0[:], in_=mask_d[:])
                make_identity(nc, ident[:])
                nc.gpsimd.memset(vtm[:, :, 128:129], 1.0)
                for dq in range(GROUP):
                    nc.sync.dma_start(
                        out=wq_s[:, ts(dq, DC), :], in_=wq_d[:, ts(dq, DC), :]
                    )
                vT = vtp.tile([128, NT], bf16)

                for tt in range(NTT):
                    if tt + 2 < NTT:  # prefetch x two tiles ahead
                        for c in range(NXC):
                            xc = xpool.tile([128, XC, TOK], bf16, tag="xt")
                            xch[(tt + 2, c)] = xc
                            nc.sync.dma_start(
                                out=xc[:],
                                in_=xt_d[:, c * XC : (c + 1) * XC, ts(tt + 2, TOK)],
                            )

                    def xsrc(Dc):
                        return xch[(tt, Dc // XC)][:, Dc % XC, :]

                    ps = ps1.tile([128, TOK], f32)
                    for Dc in range(DC):
                        nc.tensor.matmul(
                            ps[:],
                            lhsT=wk_s[:, Dc, :],
                            rhs=xsrc(Dc),
                            start=(Dc == 0),
                            stop=(Dc == DC - 1),
                        )
                    nc.scalar.activation(
                        out=kT[:, ts(tt, TOK)],
                        in_=ps[:],
                        func=AF.Identity,
                        bias=bk_s[:, 0:1],
                    )
                    ps = ps1.tile([128, TOK], f32)
                    for Dc in range(DC):
                        nc.tensor.matmul(
                            ps[:],
                            lhsT=wv_s[:, Dc, :],
                            rhs=xsrc(Dc),
                            start=(Dc == 0),
                            stop=(Dc == DC - 1),
                        )
                    nc.scalar.activation(
                        out=vT[:, ts(tt, TOK)],
                        in_=ps[:],
                        func=AF.Identity,
                        bias=bv_s[:, 0:1],
                    )
                    for dq in range(GROUP):
                        ps = ps1.tile([128, TOK], f32)
                        for Dc in range(DC):
                            nc.tensor.matmul(
                                ps[:],
                                lhsT=wq_s[:, dq * DC + Dc, :],
                                rhs=xsrc(Dc),
                                start=(Dc == 0),
                                stop=(Dc == DC - 1),
                            )
                        nc.scalar.activation(
                            out=qT[:, dq, ts(tt, TOK)],
                            in_=ps[:],
                            func=AF.Identity,
                            bias=bq_s[:, dq : dq + 1],
                        )

                # transpose v to token-major tiles (ones col already set)
                for t in range(NT // 128):
                    pt = pstr.tile([128, 128], bf16)
                    nc.tensor.transpose(pt[:], vT[:, ts(t, 128)], ident[:])
                    nc.vector.tensor_copy(out=vtm[:, t, 0:128], in_=pt[:])

            # ---------------- Phase 2: causal attention ----------------
            with tc.tile_pool(name="wout", bufs=1) as wop:
                wo_s = wop.tile([128, GROUP * DC, 128], bf16)
                nc.sync.dma_start(out=wo_s[:], in_=wo_d[:])

                with (
                    tc.psum_pool(name="att_st", bufs=2) as stp,
                    tc.psum_pool(name="att_y", bufs=4) as yqp,
                    tc.tile_pool(name="ptile", bufs=4) as ppool,
                    tc.tile_pool(name="invl", bufs=8) as invp,
                ):
                    for bh in range(B * GROUP):
                        b, h = divmod(bh, GROUP)
                        for qi in range(T // TOK):
                            njt = 4 * (qi + 1)  # causal k-tiles of 128
                            yqa = yqp.tile([128, 2, 129], f32, tag="yq")
                            yqb = yqp.tile([128, 2, 129], f32, tag="yq")
                            yqt = [yqa, yqb]

                            def ysub(s):
                                return yqt[s // 2][:, s % 2, :]

                            for jp in range(njt // 2):
                                st = stp.tile([128, 2, TOK], f32)
                                nlo_pair = max(2 * jp - 4 * qi, 0) * 128
                                for jj in range(2):
                                    j = jp * 2 + jj
                                    r = j - 4 * qi
                                    nlo = max(r, 0) * 128
                                    diag = r >= 0
                                    nc.tensor.matmul(
                                        st[:, jj, nlo:TOK],
                                        lhsT=kT[:, ds(b * T + j * 128, 128)],
                                        rhs=qT[
                                            :,
                                            h,
                                            ds(b * T + qi * TOK + nlo, TOK - nlo),
                                        ],
                                        start=True,
                                        stop=not diag,
                                        skip_group_check=diag,
                                    )
                                    if diag:
                                        # within-block causal triangle only
                                        nc.tensor.matmul(
                                            st[:, jj, nlo : nlo + 128],
                                            lhsT=ident[:],
                                            rhs=mask0[:],
                                            start=False,
                                            stop=True,
                                            skip_group_check=True,
                                        )
                                ptile = ppool.tile([128, 2, TOK], bf16)
                                nc.scalar.activation(
                                    out=ptile[:, :, nlo_pair:TOK],
                                    in_=st[:, :, nlo_pair:TOK],
                                    func=AF.Exp,
                                )
                                for jj in range(2):
                                    j = jp * 2 + jj
                                    for s in range(4):
                                        if j > 4 * qi + s:
                                            continue
                                        # start=True clears has_written for the
                                        # WHOLE psum bank, so only the first
                                        # group per bank (s even) may set it;
                                        # the odd-s group's first matmul relies
                                        # on overwrite-where-bit-clear.
                                        nc.tensor.matmul(
                                            ysub(s),
                                            lhsT=ptile[:, jj, ts(s, 128)],
                                            rhs=vtm[:, b * KT_PER_B + j, :],
                                            start=(j == 0 and s % 2 == 0),
                                            stop=(j == 4 * qi + s),
                                            skip_group_check=(
                                                j == 0 and s % 2 == 1
                                            ),
                                        )
                            for s in range(4):
                                u = bh * KT_PER_B + qi * 4 + s
                                inv = invp.tile([128, 1], f32)
                                nc.vector.reciprocal(
                                    out=inv[:], in_=ysub(s)[:, 128:129]
                                )
                                nc.vector.tensor_scalar_mul(
                                    out=yq_all[:, u, :],
                                    in0=ysub(s)[:, 0:128],
                                    scalar1=inv[:],
                                )
                                nc.vector.tensor_copy(
                                    out=l_stage[:, u : u + 1],
                                    in_=ysub(s)[:, 128:129],
                                )

                # ---------------- Phase 3: y transpose + out-projection ----------------
                with (
                    tc.psum_pool(name="ps3", bufs=4) as ps3,
                    tc.psum_pool(name="ptr", bufs=4) as ptr,
                    tc.tile_pool(name="yt", bufs=3) as ytp,
                    tc.tile_pool(name="stg", bufs=6) as stg,
                ):
                    for tt in range(NTT):
                        b, qi = divmod(tt, T // TOK)
                        yTt = ytp.tile([128, GROUP, TOK], bf16)
                        for c in range(GROUP):
                            for s in range(4):
                                u = (b * GROUP + c) * KT_PER_B + qi * 4 + s
                                pt = ptr.tile([128, 128], bf16)
                                nc.tensor.transpose(
                                    pt[:], yq_all[:, u, :], ident[:]
                                )
                                nc.vector.tensor_copy(
                                    out=yTt[:, c, ts(s, 128)], in_=pt[:]
                                )
                        for Do in range(DC):
                            ps = ps3.tile([128, TOK], f32)
                            for c in range(GROUP):
                                nc.tensor.matmul(
                                    ps[:],
                                    lhsT=wo_s[:, c * DC + Do, :],
                                    rhs=yTt[:, c, :],
                                    start=(c == 0),
                                    stop=(c == GROUP - 1),
                                )
                            so = stg.tile([128, TOK], bf16)
                            # alternate copies between DVE and ACT so neither
                            # becomes the phase bottleneck
                            if Do % 2 == 0:
                                nc.vector.tensor_copy(out=so[:], in_=ps[:])
                            else:
                                nc.scalar.copy(out=so[:], in_=ps[:])
                            nc.sync.dma_start(
                                out=out_d[:, Do, ts(tt, TOK)], in_=so[:]
                            )
                    nc.sync.dma_start(out=dbgy_d[:], in_=yq_all[:])
                    nc.sync.dma_start(out=dbgl_d[:], in_=l_stage[:])

    if not nc.is_finalized():
        nc.finalize()
    return nc


def _prep_inputs(hidden_states, Wq, bq, Wk, bk, Wv, bv, Wo, bo):
    scale = 1.0 / math.sqrt(d)

    x_flat = np.asarray(hidden_states, dtype=np.float32).reshape(NT, D)
    # xt[p, Dc, t] = x[t, Dc*128+p]
    xt = np.ascontiguousarray(
        x_flat.reshape(NT, DC, 128).transpose(2, 1, 0)
    ).astype(BF16)

    jj = np.arange(128)[:, None]
    ii = np.arange(128)[None, :]
    mask0 = np.where(jj > ii, np.float32(-1e9), np.float32(0.0)).astype(BF16)
    mask0 = np.ascontiguousarray(mask0)

    in_maps = []
    for g in range(NC_):
        Wq_g = np.asarray(Wq[g * 512 : (g + 1) * 512, :], dtype=np.float32) * scale
        bq_g = np.asarray(bq[g * 512 : (g + 1) * 512], dtype=np.float32) * scale
        Wk_g = np.asarray(Wk[g * 128 : (g + 1) * 128, :], dtype=np.float32)
        bk_g = np.asarray(bk[g * 128 : (g + 1) * 128], dtype=np.float32)
        Wv_g = np.asarray(Wv[g * 128 : (g + 1) * 128, :], dtype=np.float32)
        bv_g = np.asarray(bv[g * 128 : (g + 1) * 128], dtype=np.float32)
        Wo_g = np.asarray(Wo[:, g * 512 : (g + 1) * 512], dtype=np.float32)

        # wq[p, dq*DC+Dc, m] = Wq_g[dq*128+m, Dc*128+p]
        wq_host = np.ascontiguousarray(
            Wq_g.reshape(GROUP, 128, DC, 128).transpose(3, 0, 2, 1).reshape(
                128, GROUP * DC, 128
            )
        ).astype(BF16)
        # wk[p, Dc, m] = Wk_g[m, Dc*128+p]
        wk_host = np.ascontiguousarray(
            Wk_g.reshape(128, DC, 128).transpose(2, 1, 0)
        ).astype(BF16)
        wv_host = np.ascontiguousarray(
            Wv_g.reshape(128, DC, 128).transpose(2, 1, 0)
        ).astype(BF16)
        # wo[p, c*DC+Do, m] = Wo_g[Do*128+m, c*128+p]
        wo_host = np.ascontiguousarray(
            Wo_g.reshape(DC, 128, GROUP, 128).transpose(3, 2, 0, 1).reshape(
                128, GROUP * DC, 128
            )
        ).astype(BF16)

        in_maps.append(
            {
                "xt": xt,
                "wq": wq_host,
                "wk": wk_host,
                "wv": wv_host,
                "wo": wo_host,
                "bq": np.ascontiguousarray(bq_g.reshape(GROUP, 128).T),
                "bk": bk_g.reshape(128, 1).copy(),
                "bv": bv_g.reshape(128, 1).copy(),
                "mask0": mask0,
            }
        )
    return in_maps


def kernel(
    hidden_states, Wq, bq, Wk, bk, Wv, bv, Wo, bo, _trace=False, _result_box=None
):
    from concourse.bass_utils import run_bass_kernel_spmd

    if "nc" not in _program_cache:
        _program_cache["nc"] = _build_program()
    nc = _program_cache["nc"]

    in_maps = _prep_inputs(hidden_states, Wq, bq, Wk, bk, Wv, bv, Wo, bo)
    res = run_bass_kernel_spmd(
        nc, in_maps, core_ids=list(range(NC_)), trace=_trace
    )
    if _result_box is not None:
        _result_box.append(res)

    acc = np.zeros((128, DC, NT), dtype=np.float32)
    for r in res.results:
        acc += np.asarray(r["out"], dtype=np.float32)
    # outT[Do*128+p, t] = acc[p, Do, t];  out[t, :] = outT[:, t] + bo
    outT = acc.transpose(1, 0, 2).reshape(D, NT)
    out = outT.T + np.asarray(bo, dtype=np.float32)[None, :]
    return np.ascontiguousarray(out.reshape(B, T, D), dtype=np.float32)


# revision 10
# speedup vs baseline: 1.0035x; 1.0035x over previous
"""GQA attention (B=2, T=2048, D=4096, H=32, G=8, d=128) on 8 TRN2 NeuronCores.

Sharding: one KV group per core (4 Q heads + 1 K/V head). Each core:
  - projects q/k/v for its group in transposed ("d-major") layout,
  - causal attention with transposed score tiles S.T = K.T-chunk @ Q-tile,
    exp on ACT; the P@V matmul is flipped (stationary = P chunk, moving =
    V tile with a ones-column appended) so y comes out token-major AND the
    softmax row-sums accumulate for free in PSUM column 128,
  - normalization is then a per-partition reciprocal+scale on DVE,
  - y tiles are transposed back to d-major at the start of phase 3,
    pipelined with the partial out-projection against 512 columns of Wo.
Host sums the 8 bf16 partial outputs in f32 and adds bo.

All matmuls in bf16 with fp32 PSUM accumulation. Score tiles on the causal
diagonal are narrowed to the valid trapezoid (matmul N, exp width).
"""

import math
import sys

import numpy as np

sys.path.insert(0, "/opt/trn_rl_repo")

import ml_dtypes

BF16 = ml_dtypes.bfloat16

B, T, D = 2, 2048, 4096
H, G, d = 32, 8, 128
GROUP = H // G  # 4 heads per group/core
NT = B * T  # 4096 tokens
NC_ = 8  # cores

TOK = 512  # q-token tile (free dim of score matmuls, psum bank)
NTT = NT // TOK  # 8
DC = D // 128  # 32 contraction chunks
XC = 8  # Dc chunks per x DMA chunk (1MB each)
NXC = DC // XC  # 4 x-chunks per token tile
KT_PER_B = T // 128  # 16 k-tiles per batch

_program_cache = {}


def _build_program():
    import concourse.mybir as mybir
    import concourse.tile as tile
    from concourse import bacc
    from concourse.bass import ds, ts
    from concourse.masks import make_identity

    f32 = mybir.dt.float32
    bf16 = mybir.dt.bfloat16
    AF = mybir.ActivationFunctionType

    nc = bacc.Bacc()

    xt_d = nc.declare_dram_parameter("xt", [128, DC, NT], bf16, isOutput=False)
    wq_d = nc.declare_dram_parameter("wq", [128, DC * GROUP, 128], bf16, isOutput=False)
    wk_d = nc.declare_dram_parameter("wk", [128, DC, 128], bf16, isOutput=False)
    wv_d = nc.declare_dram_parameter("wv", [128, DC, 128], bf16, isOutput=False)
    wo_d = nc.declare_dram_parameter("wo", [128, GROUP * DC, 128], bf16, isOutput=False)
    bq_d = nc.declare_dram_parameter("bq", [128, GROUP], f32, isOutput=False)
    bk_d = nc.declare_dram_parameter("bk", [128, 1], f32, isOutput=False)
    bv_d = nc.declare_dram_parameter("bv", [128, 1], f32, isOutput=False)
    mask_d = nc.declare_dram_parameter("mask0", [128, 128], bf16, isOutput=False)
    out_d = nc.declare_dram_parameter("out", [128, DC, NT], bf16, isOutput=True)

    with tile.TileContext(nc) as tc:
        with tc.tile_pool(name="persist", bufs=1) as persist:
            qT = persist.tile([128, GROUP, NT], bf16)  # [dq_row, head, tok]
            kT = persist.tile([128, NT], bf16)  # [d, tok]
            # v token-major tiles + ones column for free softmax row-sums
            vtm = persist.tile([128, NT // 128, 129], bf16)
            # y token-major: [tok%128, (b,h,qi,s), dv]
            yq_all = persist.tile([128, B * GROUP * (T // 128), 128], bf16)
            mask0 = persist.tile([128, 128], bf16)
            bq_s = persist.tile([128, GROUP], f32)
            bk_s = persist.tile([128, 1], f32)
            bv_s = persist.tile([128, 1], f32)
            ident = persist.tile([128, 128], bf16)

            # ---------------- Phase 1: q/k/v projections ----------------
            with (
                tc.tile_pool(name="wproj", bufs=1) as wpool,
                tc.tile_pool(name="xtp", bufs=8) as xpool,
                tc.tile_pool(name="vtstage", bufs=1) as vtp,
                tc.psum_pool(name="ps1", bufs=4) as ps1,
                tc.psum_pool(name="pstr", bufs=2) as pstr,
            ):
                wk_s = wpool.tile([128, DC, 128], bf16)
                wv_s = wpool.tile([128, DC, 128], bf16)
                wq_s = wpool.tile([128, GROUP * DC, 128], bf16)

                # critical-path-first DMA ordering: the very first matmul
                # needs only wk chunk 0 + x chunk (0,0).
                nc.sync.dma_start(out=bk_s[:], in_=bk_d[:])
                nc.sync.dma_start(out=wk_s[:, 0:XC, :], in_=wk_d[:, 0:XC, :])
                xch = {}
                for tt in range(2):  # prefetch two token tiles of x
                    for c in range(NXC):
                        xc = xpool.tile([128, XC, TOK], bf16, tag="xt")
                        xch[(tt, c)] = xc
                        nc.sync.dma_start(
                            out=xc[:], in_=xt_d[:, c * XC : (c + 1) * XC, ts(tt, TOK)]
                        )
                        if tt == 0 and c == 0:
                            for cc in range(1, NXC):
                                nc.sync.dma_start(
                                    out=wk_s[:, cc * XC : (cc + 1) * XC, :],
                                    in_=wk_d[:, cc * XC : (cc + 1) * XC, :],
                                )
                nc.sync.dma_start(out=bv_s[:], in_=bv_d[:])
                nc.sync.dma_start(out=bq_s[:], in_=bq_d[:])
                nc.sync.dma_start(out=wv_s[:], in_=wv_d[:])
                nc.sync.dma_start(out=mask0[:], in_=mask_d[:])
                make_identity(nc, ident[:])
                nc.gpsimd.memset(vtm[:, :, 128:129], 1.0)
                for dq in range(GROUP):
                    nc.sync.dma_start(
                        out=wq_s[:, ts(dq, DC), :], in_=wq_d[:, ts(dq, DC), :]
                    )
                vT = vtp.tile([128, NT], bf16)

                for tt in range(NTT):
                    if tt + 2 < NTT:  # prefetch x two tiles ahead
                        for c in range(NXC):
                            xc = xpool.tile([128, XC, TOK], bf16, tag="xt")
                            xch[(tt + 2, c)] = xc
                            nc.sync.dma_start(
                                out=xc[:],
                                in_=xt_d[:, c * XC : (c + 1) * XC, ts(tt + 2, TOK)],
                            )

                    def xsrc(Dc):
                        return xch[(tt, Dc // XC)][:, Dc % XC, :]

                    ps = ps1.tile([128, TOK], f32)
                    for Dc in range(DC):
                        nc.tensor.matmul(
                            ps[:],
                            lhsT=wk_s[:, Dc, :],
                            rhs=xsrc(Dc),
                            start=(Dc == 0),
                            stop=(Dc == DC - 1),
                        )
                    nc.scalar.activation(
                        out=kT[:, ts(tt, TOK)],
                        in_=ps[:],
                        func=AF.Identity,
                        bias=bk_s[:, 0:1],
                    )
                    ps = ps1.tile([128, TOK], f32)
                    for Dc in range(DC):
                        nc.tensor.matmul(
                            ps[:],
                            lhsT=wv_s[:, Dc, :],
                            rhs=xsrc(Dc),
                            start=(Dc == 0),
                            stop=(Dc == DC - 1),
                        )
                    nc.scalar.activation(
                        out=vT[:, ts(tt, TOK)],
                        in_=ps[:],
                        func=AF.Identity,
                        bias=bv_s[:, 0:1],
                    )
                    for dq in range(GROUP):
                        ps = ps1.tile([128, TOK], f32)
                        for Dc in range(DC):
                            nc.tensor.matmul(
                                ps[:],
                                lhsT=wq_s[:, dq * DC + Dc, :],
                                rhs=xsrc(Dc),
                                start=(Dc == 0),
                                stop=(Dc == DC - 1),
                            )
                        nc.scalar.activation(
                            out=qT[:, dq, ts(tt, TOK)],
                            in_=ps[:],
                            func=AF.Identity,
                            bias=bq_s[:, dq : dq + 1],
                        )

                # transpose v to token-major tiles (ones col already set)
                for t in range(NT // 128):
                    pt = pstr.tile([128, 128], bf16)
                    nc.tensor.transpose(pt[:], vT[:, ts(t, 128)], ident[:])
                    nc.vector.tensor_copy(out=vtm[:, t, 0:128], in_=pt[:])

            # ---------------- Phase 2: causal attention ----------------
            with tc.tile_pool(name="wout", bufs=1) as wop:
                wo_s = wop.tile([128, GROUP * DC, 128], bf16)
                nc.sync.dma_start(out=wo_s[:], in_=wo_d[:])

                with (
                    tc.psum_pool(name="att_st", bufs=2) as stp,
                    tc.psum_pool(name="att_y", bufs=4) as yqp,
                    tc.tile_pool(name="ptile", bufs=4) as ppool,
                    tc.tile_pool(name="invl", bufs=8) as invp,
                ):
                    for bh in range(B * GROUP):
                        b, h = divmod(bh, GROUP)
                        for qi in range(T // TOK):
                            njt = 4 * (qi + 1)  # causal k-tiles of 128
                            yqa = yqp.tile([128, 2, 129], f32, tag="yq")
                            yqb = yqp.tile([128, 2, 129], f32, tag="yq")
                            yqt = [yqa, yqb]

                            def ysub(s):
                                return yqt[s // 2][:, s % 2, :]

                            for jp in range(njt // 2):
                                st = stp.tile([128, 2, TOK], f32)
                                nlo_pair = max(2 * jp - 4 * qi, 0) * 128
                                for jj in range(2):
                                    j = jp * 2 + jj
                                    r = j - 4 * qi
                                    nlo = max(r, 0) * 128
                                    diag = r >= 0
                                    nc.tensor.matmul(
                                        st[:, jj, nlo:TOK],
                                        lhsT=kT[:, ds(b * T + j * 128, 128)],
                                        rhs=qT[
                                            :,
                                            h,
                                            ds(b * T + qi * TOK + nlo, TOK - nlo),
                                        ],
                                        start=True,
                                        stop=not diag,
                                        skip_group_check=diag,
                                    )
                                    if diag:
                                        # within-block causal triangle only
                                        nc.tensor.matmul(
                                            st[:, jj, nlo : nlo + 128],
                                            lhsT=ident[:],
                                            rhs=mask0[:],
                                            start=False,
                                            stop=True,
                                            skip_group_check=True,
                                        )
                                ptile = ppool.tile([128, 2, TOK], bf16)
                                nc.scalar.activation(
                                    out=ptile[:, :, nlo_pair:TOK],
                                    in_=st[:, :, nlo_pair:TOK],
                                    func=AF.Exp,
                                )
                                for jj in range(2):
                                    j = jp * 2 + jj
                                    for s in range(4):
                                        if j > 4 * qi + s:
                                            continue
                                        # start=True clears has_written for the
                                        # WHOLE psum bank, so only the first
                                        # group per bank (s even) may set it;
                                        # the odd-s group's first matmul relies
                                        # on overwrite-where-bit-clear.
                                        nc.tensor.matmul(
                                            ysub(s),
                                            lhsT=ptile[:, jj, ts(s, 128)],
                                            rhs=vtm[:, b * KT_PER_B + j, :],
                                            start=(j == 0 and s % 2 == 0),
                                            stop=(j == 4 * qi + s),
                                            skip_group_check=(
                                                j == 0 and s % 2 == 1
                                            ),
                                        )
                            for s in range(4):
                                u = bh * KT_PER_B + qi * 4 + s
                                inv = invp.tile([128, 1], f32)
                                nc.vector.reciprocal(
                                    out=inv[:], in_=ysub(s)[:, 128:129]
                                )
                                nc.vector.tensor_scalar_mul(
                                    out=yq_all[:, u, :],
                                    in0=ysub(s)[:, 0:128],
                                    scalar1=inv[:],
                                )

                # ---------------- Phase 3: y transpose + out-projection ----------------
                with (
                    tc.psum_pool(name="ps3", bufs=4) as ps3,
                    tc.psum_pool(name="ptr", bufs=4) as ptr,
                    tc.tile_pool(name="yt", bufs=3) as ytp,
                    tc.tile_pool(name="stg", bufs=6) as stg,
                ):
                    for tt in range(NTT):
                        b, qi = divmod(tt, T // TOK)
                        yTt = ytp.tile([128, GROUP, TOK], bf16)
                        for c in range(GROUP):
                            for s in range(4):
                                u = (b * GROUP + c) * KT_PER_B + qi * 4 + s
                                pt = ptr.tile([128, 128], bf16)
                                nc.tensor.transpose(
                                    pt[:], yq_all[:, u, :], ident[:]
                                )
                                nc.vector.tensor_copy(
                                    out=yTt[:, c, ts(s, 128)], in_=pt[:]
                                )
                        for Do in range(DC):
                            ps = ps3.tile([128, TOK], f32)
                            for c in range(GROUP):
                                nc.tensor.matmul(
                                    ps[:],
                                    lhsT=wo_s[:, c * DC + Do, :],
                                    rhs=yTt[:, c, :],
                                    start=(c == 0),
                                    stop=(c == GROUP - 1),
                                )
                            so = stg.tile([128, TOK], bf16)
                            # alternate copies between DVE and ACT so neither
                            # becomes the phase bottleneck
                            if Do % 2 == 0:
                                nc.vector.tensor_copy(out=so[:], in_=ps[:])
                            else:
                                nc.scalar.copy(out=so[:], in_=ps[:])
                            nc.sync.dma_start(
                                out=out_d[:, Do, ts(tt, TOK)], in_=so[:]
                            )

    if not nc.is_finalized():
        nc.finalize()
    return nc


def _prep_inputs(hidden_states, Wq, bq, Wk, bk, Wv, bv, Wo, bo):
    scale = 1.0 / math.sqrt(d)

    x_flat = np.asarray(hidden_states, dtype=np.float32).reshape(NT, D)
    # xt[p, Dc, t] = x[t, Dc*128+p]
    xt = np.ascontiguousarray(
        x_flat.reshape(NT, DC, 128).transpose(2, 1, 0)
    ).astype(BF16)

    jj = np.arange(128)[:, None]
    ii = np.arange(128)[None, :]
    mask0 = np.where(jj > ii, np.float32(-1e9), np.float32(0.0)).astype(BF16)
    mask0 = np.ascontiguousarray(mask0)

    in_maps = []
    for g in range(NC_):
        Wq_g = np.asarray(Wq[g * 512 : (g + 1) * 512, :], dtype=np.float32) * scale
        bq_g = np.asarray(bq[g * 512 : (g + 1) * 512], dtype=np.float32) * scale
        Wk_g = np.asarray(Wk[g * 128 : (g + 1) * 128, :], dtype=np.float32)
        bk_g = np.asarray(bk[g * 128 : (g + 1) * 128], dtype=np.float32)
        Wv_g = np.asarray(Wv[g * 128 : (g + 1) * 128, :], dtype=np.float32)
        bv_g = np.asarray(bv[g * 128 : (g + 1) * 128], dtype=np.float32)
        Wo_g = np.asarray(Wo[:, g * 512 : (g + 1) * 512], dtype=np.float32)

        # wq[p, dq*DC+Dc, m] = Wq_g[dq*128+m, Dc*128+p]
        wq_host = np.ascontiguousarray(
            Wq_g.reshape(GROUP, 128, DC, 128).transpose(3, 0, 2, 1).reshape(
                128, GROUP * DC, 128
            )
        ).astype(BF16)
        # wk[p, Dc, m] = Wk_g[m, Dc*128+p]
        wk_host = np.ascontiguousarray(
            Wk_g.reshape(128, DC, 128).transpose(2, 1, 0)
        ).astype(BF16)
        wv_host = np.ascontiguousarray(
            Wv_g.reshape(128, DC, 128).transpose(2, 1, 0)
        ).astype(BF16)
        # wo[p, c*DC+Do, m] = Wo_g[Do*128+m, c*128+p]
        wo_host = np.ascontiguousarray(
            Wo_g.reshape(DC, 128, GROUP, 128).transpose(3, 2, 0, 1).reshape(
                128, GROUP * DC, 128
            )
        ).astype(BF16)

        in_maps.append(
            {
                "xt": xt,
                "wq": wq_host,
                "wk": wk_host,
                "wv": wv_host,
                "wo": wo_host,
                "bq": np.ascontiguousarray(bq_g.reshape(GROUP, 128).T),
                "bk": bk_g.reshape(128, 1).copy(),
                "bv": bv_g.reshape(128, 1).copy(),
                "mask0": mask0,
            }
        )
    return in_maps


def kernel(
    hidden_states, Wq, bq, Wk, bk, Wv, bv, Wo, bo, _trace=False, _result_box=None
):
    from concourse.bass_utils import run_bass_kernel_spmd

    if "nc" not in _program_cache:
        _program_cache["nc"] = _build_program()
    nc = _program_cache["nc"]

    in_maps = _prep_inputs(hidden_states, Wq, bq, Wk, bk, Wv, bv, Wo, bo)
    res = run_bass_kernel_spmd(
        nc, in_maps, core_ids=list(range(NC_)), trace=_trace
    )
    if _result_box is not None:
        _result_box.append(res)

    acc = np.zeros((128, DC, NT), dtype=np.float32)
    for r in res.results:
        acc += np.asarray(r["out"], dtype=np.float32)
    # outT[Do*128+p, t] = acc[p, Do, t];  out[t, :] = outT[:, t] + bo
    outT = acc.transpose(1, 0, 2).reshape(D, NT)
    out = outT.T + np.asarray(bo, dtype=np.float32)[None, :]
    return np.ascontiguousarray(out.reshape(B, T, D), dtype=np.float32)


# revision 11
# speedup vs baseline: 1.1680x; 1.1640x over previous
"""GQA attention (B=2, T=2048, D=4096, H=32, G=8, d=128) on 8 TRN2 NeuronCores.

Sharding: one KV group per core (4 Q heads + 1 K/V head). Each core:
  - projects q/k/v for its group in transposed ("d-major") layout,
  - causal attention with transposed score tiles S.T = K.T-chunk @ Q-tile.
    Softmax row-sums come from a bf16 running sum of the exp'd P tiles on
    DVE (4x-mode scalar_tensor_tensor) + one ones-matmul pair per chain —
    replacing the per-k-tile ones-matmuls of the naive version.  Score
    matmuls on the causal diagonal are narrowed to the valid trapezoid;
    the masked left region is filled via the mask matmul exploiting the
    psum has_written overwrite-where-clear semantics.
  - partial out-projection against its 512 columns of Wo, bf16 partials.
Host sums the 8 partial outputs in f32 and adds bo.

All matmuls in bf16 with fp32 PSUM accumulation.
"""

import math
import sys

import numpy as np

sys.path.insert(0, "/opt/trn_rl_repo")

import ml_dtypes

BF16 = ml_dtypes.bfloat16

B, T, D = 2, 2048, 4096
H, G, d = 32, 8, 128
GROUP = H // G  # 4 heads per group/core
NT = B * T  # 4096 tokens
NC_ = 8  # cores

TOK = 512  # q-token tile (free dim of score matmuls, psum bank)
NTT = NT // TOK  # 8
DC = D // 128  # 32 contraction chunks
XC = 8  # Dc chunks per x DMA chunk (1MB each)
NXC = DC // XC  # 4 x-chunks per token tile
KT_PER_B = T // 128  # 16 k-tiles per batch

_program_cache = {}


def _build_program():
    import concourse.mybir as mybir
    import concourse.tile as tile
    from concourse import bacc
    from concourse.bass import ds, ts
    from concourse.masks import make_identity

    f32 = mybir.dt.float32
    bf16 = mybir.dt.bfloat16
    AF = mybir.ActivationFunctionType
    ALU = mybir.AluOpType

    nc = bacc.Bacc()

    xt_d = nc.declare_dram_parameter("xt", [128, DC, NT], bf16, isOutput=False)
    wq_d = nc.declare_dram_parameter("wq", [128, DC * GROUP, 128], bf16, isOutput=False)
    wk_d = nc.declare_dram_parameter("wk", [128, DC, 128], bf16, isOutput=False)
    wv_d = nc.declare_dram_parameter("wv", [128, DC, 128], bf16, isOutput=False)
    wo_d = nc.declare_dram_parameter("wo", [128, GROUP * DC, 128], bf16, isOutput=False)
    bq_d = nc.declare_dram_parameter("bq", [128, GROUP], f32, isOutput=False)
    bk_d = nc.declare_dram_parameter("bk", [128, 1], f32, isOutput=False)
    bv_d = nc.declare_dram_parameter("bv", [128, 1], f32, isOutput=False)
    mask_d = nc.declare_dram_parameter("masks", [128, 4, TOK], bf16, isOutput=False)
    out_d = nc.declare_dram_parameter("out", [128, DC, NT], bf16, isOutput=True)

    with tile.TileContext(nc) as tc:
        with tc.tile_pool(name="persist", bufs=1) as persist:
            qT = persist.tile([128, GROUP, NT], bf16)  # [dq_row, head, tok]
            kT = persist.tile([128, NT], bf16)  # [d, tok]
            vtm = persist.tile([128, NT // 128, 128], bf16)  # [tok%128, tile, dv]
            yT = persist.tile([128, GROUP, NT], bf16)  # [dv, head, tok]
            maskb = persist.tile([128, 4, TOK], bf16)
            bq_s = persist.tile([128, GROUP], f32)
            bk_s = persist.tile([128, 1], f32)
            bv_s = persist.tile([128, 1], f32)
            ones128 = persist.tile([128, 128], bf16)
            ident = persist.tile([128, 128], bf16)

            # ---------------- Phase 1: q/k/v projections ----------------
            with (
                tc.tile_pool(name="wproj", bufs=1) as wpool,
                tc.tile_pool(name="xtp", bufs=8) as xpool,
                tc.tile_pool(name="vtstage", bufs=1) as vtp,
                tc.psum_pool(name="ps1", bufs=4) as ps1,
                tc.psum_pool(name="pstr", bufs=2) as pstr,
            ):
                wk_s = wpool.tile([128, DC, 128], bf16)
                wv_s = wpool.tile([128, DC, 128], bf16)
                wq_s = wpool.tile([128, GROUP * DC, 128], bf16)

                # critical-path-first DMA ordering: the very first matmuls
                # need only wk chunk 0 + x chunk (0,0); q-proj of tile 0
                # needs wq head 0 before the second x tile.
                nc.sync.dma_start(out=bk_s[:], in_=bk_d[:])
                nc.sync.dma_start(out=wk_s[:, 0:XC, :], in_=wk_d[:, 0:XC, :])
                xch = {}
                for c in range(NXC):
                    xc = xpool.tile([128, XC, TOK], bf16, tag="xt")
                    xch[(0, c)] = xc
                    nc.sync.dma_start(
                        out=xc[:], in_=xt_d[:, c * XC : (c + 1) * XC, ts(0, TOK)]
                    )
                    if c == 0:
                        for cc in range(1, NXC):
                            nc.sync.dma_start(
                                out=wk_s[:, cc * XC : (cc + 1) * XC, :],
                                in_=wk_d[:, cc * XC : (cc + 1) * XC, :],
                            )
                nc.sync.dma_start(out=bv_s[:], in_=bv_d[:])
                nc.sync.dma_start(out=wv_s[:], in_=wv_d[:])
                nc.sync.dma_start(out=bq_s[:], in_=bq_d[:])
                nc.sync.dma_start(out=wq_s[:, ts(0, DC), :], in_=wq_d[:, ts(0, DC), :])
                for c in range(NXC):
                    xc = xpool.tile([128, XC, TOK], bf16, tag="xt")
                    xch[(1, c)] = xc
                    nc.sync.dma_start(
                        out=xc[:], in_=xt_d[:, c * XC : (c + 1) * XC, ts(1, TOK)]
                    )
                for dq in range(1, GROUP):
                    nc.sync.dma_start(
                        out=wq_s[:, ts(dq, DC), :], in_=wq_d[:, ts(dq, DC), :]
                    )
                nc.sync.dma_start(out=maskb[:], in_=mask_d[:])
                make_identity(nc, ident[:])
                nc.vector.memset(ones128[:], 1.0)
                vT = vtp.tile([128, NT], bf16)

                for tt in range(NTT):
                    if tt + 2 < NTT:  # prefetch x two tiles ahead
                        for c in range(NXC):
                            xc = xpool.tile([128, XC, TOK], bf16, tag="xt")
                            xch[(tt + 2, c)] = xc
                            nc.sync.dma_start(
                                out=xc[:],
                                in_=xt_d[:, c * XC : (c + 1) * XC, ts(tt + 2, TOK)],
                            )

                    def xsrc(Dc):
                        return xch[(tt, Dc // XC)][:, Dc % XC, :]

                    ps = ps1.tile([128, TOK], f32)
                    for Dc in range(DC):
                        nc.tensor.matmul(
                            ps[:],
                            lhsT=wk_s[:, Dc, :],
                            rhs=xsrc(Dc),
                            start=(Dc == 0),
                            stop=(Dc == DC - 1),
                        )
                    nc.scalar.activation(
                        out=kT[:, ts(tt, TOK)],
                        in_=ps[:],
                        func=AF.Identity,
                        bias=bk_s[:, 0:1],
                    )
                    ps = ps1.tile([128, TOK], f32)
                    for Dc in range(DC):
                        nc.tensor.matmul(
                            ps[:],
                            lhsT=wv_s[:, Dc, :],
                            rhs=xsrc(Dc),
                            start=(Dc == 0),
                            stop=(Dc == DC - 1),
                        )
                    nc.scalar.activation(
                        out=vT[:, ts(tt, TOK)],
                        in_=ps[:],
                        func=AF.Identity,
                        bias=bv_s[:, 0:1],
                    )
                    for dq in range(GROUP):
                        ps = ps1.tile([128, TOK], f32)
                        for Dc in range(DC):
                            nc.tensor.matmul(
                                ps[:],
                                lhsT=wq_s[:, dq * DC + Dc, :],
                                rhs=xsrc(Dc),
                                start=(Dc == 0),
                                stop=(Dc == DC - 1),
                            )
                        nc.scalar.activation(
                            out=qT[:, dq, ts(tt, TOK)],
                            in_=ps[:],
                            func=AF.Identity,
                            bias=bq_s[:, dq : dq + 1],
                        )

                # transpose v to token-major tiles
                for t in range(NT // 128):
                    pt = pstr.tile([128, 128], bf16)
                    nc.tensor.transpose(pt[:], vT[:, ts(t, 128)], ident[:])
                    nc.vector.tensor_copy(out=vtm[:, t, :], in_=pt[:])

            # ---------------- Phase 2: causal attention ----------------
            with tc.tile_pool(name="wout", bufs=1) as wop:
                wo_s = wop.tile([128, GROUP * DC, 128], bf16)
                nc.sync.dma_start(out=wo_s[:], in_=wo_d[:])

                with (
                    tc.psum_pool(name="att_st", bufs=2) as stp,
                    tc.psum_pool(name="att_y", bufs=2) as yp,
                    tc.psum_pool(name="att_l", bufs=2) as lpool,
                    tc.tile_pool(name="ptile", bufs=6) as ppool,
                    tc.tile_pool(name="acc", bufs=3) as accp,
                    tc.tile_pool(name="invl", bufs=2) as invp,
                ):
                    for bh in range(B * GROUP):
                        b, h = divmod(bh, GROUP)
                        for qi in range(T // TOK):
                            njt = 4 * (qi + 1)  # causal k-tiles of 128
                            yps = yp.tile([128, TOK], f32)
                            accEO = accp.tile([128, 2, TOK], bf16)
                            for jp in range(njt // 2):
                                st = stp.tile([128, 2, TOK], f32)
                                for jj in range(2):
                                    j = jp * 2 + jj
                                    r = j - 4 * qi
                                    diag = r >= 0
                                    nlo = max(r, 0) * 128
                                    nc.tensor.matmul(
                                        st[:, jj, nlo:TOK],
                                        lhsT=kT[:, ds(b * T + j * 128, 128)],
                                        rhs=qT[
                                            :,
                                            h,
                                            ds(b * T + qi * TOK + nlo, TOK - nlo),
                                        ],
                                        start=True,
                                        stop=not diag,
                                        skip_group_check=diag,
                                    )
                                    if diag:
                                        # left cols [0,nlo) have has_written
                                        # clear -> mask OVERWRITES them with
                                        # -1e9; [nlo,nlo+128) accumulates the
                                        # causal triangle onto the scores.
                                        nc.tensor.matmul(
                                            st[:, jj, 0 : nlo + 128],
                                            lhsT=ident[:],
                                            rhs=maskb[:, r, 0 : nlo + 128],
                                            start=False,
                                            stop=True,
                                            skip_group_check=True,
                                        )
                                ptile = ppool.tile([128, 2, TOK], bf16)
                                nc.scalar.activation(
                                    out=ptile[:], in_=st[:], func=AF.Exp
                                )
                                # running bf16 sum of P tiles (softmax denom)
                                if jp == 0:
                                    nc.vector.tensor_copy(
                                        out=accEO[:], in_=ptile[:]
                                    )
                                else:
                                    nc.vector.scalar_tensor_tensor(
                                        out=accEO[:],
                                        in0=ptile[:],
                                        scalar=1.0,
                                        in1=accEO[:],
                                        op0=ALU.mult,
                                        op1=ALU.add,
                                    )
                                for jj in range(2):
                                    j = jp * 2 + jj
                                    nlo = max(j - 4 * qi, 0) * 128
                                    nc.tensor.matmul(
                                        yps[:, nlo:TOK],
                                        lhsT=vtm[:, b * KT_PER_B + j, :],
                                        rhs=ptile[:, jj, nlo:TOK],
                                        start=(j == 0),
                                        stop=(j == njt - 1),
                                        skip_group_check=(
                                            nlo > 0 or j == njt - 1
                                        ),
                                    )
                            # row sums: one ones-matmul pair per chain
                            lps = lpool.tile([128, TOK], f32)
                            nc.tensor.matmul(
                                lps[:],
                                lhsT=ones128[:],
                                rhs=accEO[:, 0, :],
                                start=True,
                                stop=False,
                            )
                            nc.tensor.matmul(
                                lps[:],
                                lhsT=ones128[:],
                                rhs=accEO[:, 1, :],
                                start=False,
                                stop=True,
                            )
                            inv = invp.tile([128, TOK], f32)
                            scratch = invp.tile([128, TOK], f32, tag="rscr")
                            nc.vector.reciprocal_approx_accurate(
                                out=inv[:], in_=lps[:], scratch=scratch[:]
                            )
                            nc.vector.tensor_mul(
                                out=yT[:, h, ds(b * T + qi * TOK, TOK)],
                                in0=yps[:],
                                in1=inv[:],
                            )

                # ---------------- Phase 3: partial out-projection ----------------
                with (
                    tc.psum_pool(name="ps3", bufs=6) as ps3,
                    tc.tile_pool(name="stg", bufs=6) as stg,
                ):
                    for Do in range(DC):
                        for tt in range(NTT):
                            ps = ps3.tile([128, TOK], f32)
                            for c in range(GROUP):
                                nc.tensor.matmul(
                                    ps[:],
                                    lhsT=wo_s[:, c * DC + Do, :],
                                    rhs=yT[:, c, ts(tt, TOK)],
                                    start=(c == 0),
                                    stop=(c == GROUP - 1),
                                )
                            so = stg.tile([128, TOK], bf16)
                            # alternate copies between DVE and ACT so neither
                            # becomes the phase bottleneck
                            if tt % 2 == 0:
                                nc.vector.tensor_copy(out=so[:], in_=ps[:])
                            else:
                                nc.scalar.copy(out=so[:], in_=ps[:])
                            nc.sync.dma_start(
                                out=out_d[:, Do, ts(tt, TOK)], in_=so[:]
                            )

    if not nc.is_finalized():
        nc.finalize()
    return nc


def _prep_inputs(hidden_states, Wq, bq, Wk, bk, Wv, bv, Wo, bo):
    scale = 1.0 / math.sqrt(d)

    x_flat = np.asarray(hidden_states, dtype=np.float32).reshape(NT, D)
    # xt[p, Dc, t] = x[t, Dc*128+p]
    xt = np.ascontiguousarray(
        x_flat.reshape(NT, DC, 128).transpose(2, 1, 0)
    ).astype(BF16)

    jj = np.arange(128)[:, None, None]
    rr = np.arange(4)[None, :, None] * 128
    ii = np.arange(TOK)[None, None, :]
    masks = np.where(jj + rr > ii, np.float32(-1e9), np.float32(0.0)).astype(BF16)
    masks = np.ascontiguousarray(masks)

    in_maps = []
    for g in range(NC_):
        Wq_g = np.asarray(Wq[g * 512 : (g + 1) * 512, :], dtype=np.float32) * scale
        bq_g = np.asarray(bq[g * 512 : (g + 1) * 512], dtype=np.float32) * scale
        Wk_g = np.asarray(Wk[g * 128 : (g + 1) * 128, :], dtype=np.float32)
        bk_g = np.asarray(bk[g * 128 : (g + 1) * 128], dtype=np.float32)
        Wv_g = np.asarray(Wv[g * 128 : (g + 1) * 128], dtype=np.float32)
        bv_g = np.asarray(bv[g * 128 : (g + 1) * 128], dtype=np.float32)
        Wo_g = np.asarray(Wo[:, g * 512 : (g + 1) * 512], dtype=np.float32)

        # wq[p, dq*DC+Dc, m] = Wq_g[dq*128+m, Dc*128+p]
        wq_host = np.ascontiguousarray(
            Wq_g.reshape(GROUP, 128, DC, 128).transpose(3, 0, 2, 1).reshape(
                128, GROUP * DC, 128
            )
        ).astype(BF16)
        # wk[p, Dc, m] = Wk_g[m, Dc*128+p]
        wk_host = np.ascontiguousarray(
            Wk_g.reshape(128, DC, 128).transpose(2, 1, 0)
        ).astype(BF16)
        wv_host = np.ascontiguousarray(
            Wv_g.reshape(128, DC, 128).transpose(2, 1, 0)
        ).astype(BF16)
        # wo[p, c*DC+Do, m] = Wo_g[Do*128+m, c*128+p]
        wo_host = np.ascontiguousarray(
            Wo_g.reshape(DC, 128, GROUP, 128).transpose(3, 2, 0, 1).reshape(
                128, GROUP * DC, 128
            )
        ).astype(BF16)

        in_maps.append(
            {
                "xt": xt,
                "wq": wq_host,
                "wk": wk_host,
                "wv": wv_host,
                "wo": wo_host,
                "bq": np.ascontiguousarray(bq_g.reshape(GROUP, 128).T),
                "bk": bk_g.reshape(128, 1).copy(),
                "bv": bv_g.reshape(128, 1).copy(),
                "masks": masks,
            }
        )
    return in_maps


def kernel(
    hidden_states, Wq, bq, Wk, bk, Wv, bv, Wo, bo, _trace=False, _result_box=None
):
    from concourse.bass_utils import run_bass_kernel_spmd

    if "nc" not in _program_cache:
        _program_cache["nc"] = _build_program()
    nc = _program_cache["nc"]

    in_maps = _prep_inputs(hidden_states, Wq, bq, Wk, bk, Wv, bv, Wo, bo)
    res = run_bass_kernel_spmd(
        nc, in_maps, core_ids=list(range(NC_)), trace=_trace
    )
    if _result_box is not None:
        _result_box.append(res)

    acc = np.zeros((128, DC, NT), dtype=np.float32)
    for r in res.results:
        acc += np.asarray(r["out"], dtype=np.float32)
    # outT[Do*128+p, t] = acc[p, Do, t];  out[t, :] = outT[:, t] + bo
    outT = acc.transpose(1, 0, 2).reshape(D, NT)
    out = outT.T + np.asarray(bo, dtype=np.float32)[None, :]
    return np.ascontiguousarray(out.reshape(B, T, D), dtype=np.float32)


# revision 16
# speedup vs baseline: 1.1799x; 1.0101x over previous
"""GQA attention (B=2, T=2048, D=4096, H=32, G=8, d=128) on 8 TRN2 NeuronCores.

Sharding: one KV group per core (4 Q heads + 1 K/V head). Each core:
  - projects q/k/v for its group in transposed ("d-major") layout,
  - causal attention with transposed score tiles S.T = K.T-chunk @ Q-tile.
    Softmax row-sums come from a bf16 running sum of the exp'd P tiles on
    DVE (4x-mode scalar_tensor_tensor) + one ones-matmul pair per chain —
    replacing the per-k-tile ones-matmuls of the naive version.  Score
    matmuls on the causal diagonal are narrowed to the valid trapezoid;
    the masked left region is filled via the mask matmul exploiting the
    psum has_written overwrite-where-clear semantics.
  - partial out-projection against its 512 columns of Wo, bf16 partials.
Host sums the 8 partial outputs in f32 and adds bo.

All matmuls in bf16 with fp32 PSUM accumulation.
"""

import math
import sys

import numpy as np

sys.path.insert(0, "/opt/trn_rl_repo")

import ml_dtypes

BF16 = ml_dtypes.bfloat16

B, T, D = 2, 2048, 4096
H, G, d = 32, 8, 128
GROUP = H // G  # 4 heads per group/core
NT = B * T  # 4096 tokens
NC_ = 8  # cores

TOK = 512  # q-token tile (free dim of score matmuls, psum bank)
NTT = NT // TOK  # 8
DC = D // 128  # 32 contraction chunks
XC = 4  # Dc chunks per x DMA chunk (512KB each)
NXC = DC // XC  # 8 x-chunks per token tile
KT_PER_B = T // 128  # 16 k-tiles per batch

_program_cache = {}


def _build_program():
    import concourse.mybir as mybir
    import concourse.tile as tile
    from concourse import bacc
    from concourse.bass import ds, ts
    from concourse.masks import make_identity

    f32 = mybir.dt.float32
    bf16 = mybir.dt.bfloat16
    AF = mybir.ActivationFunctionType
    ALU = mybir.AluOpType

    nc = bacc.Bacc()

    xt_d = nc.declare_dram_parameter("xt", [128, DC, NT], bf16, isOutput=False)
    wq_d = nc.declare_dram_parameter("wq", [128, DC * GROUP, 128], bf16, isOutput=False)
    wk_d = nc.declare_dram_parameter("wk", [128, DC, 128], bf16, isOutput=False)
    wv_d = nc.declare_dram_parameter("wv", [128, DC, 128], bf16, isOutput=False)
    wo_d = nc.declare_dram_parameter("wo", [128, GROUP * DC, 128], bf16, isOutput=False)
    bq_d = nc.declare_dram_parameter("bq", [128, GROUP], f32, isOutput=False)
    bk_d = nc.declare_dram_parameter("bk", [128, 1], f32, isOutput=False)
    bv_d = nc.declare_dram_parameter("bv", [128, 1], f32, isOutput=False)
    mask_d = nc.declare_dram_parameter("masks", [128, 4, TOK], bf16, isOutput=False)
    out_d = nc.declare_dram_parameter("out", [128, DC, NT], bf16, isOutput=True)

    with tile.TileContext(nc) as tc:
        with tc.tile_pool(name="persist", bufs=1) as persist:
            qT = persist.tile([128, GROUP, NT], bf16)  # [dq_row, head, tok]
            kT = persist.tile([128, NT], bf16)  # [d, tok]
            vtm = persist.tile([128, NT // 128, 128], bf16)  # [tok%128, tile, dv]
            yT = persist.tile([128, GROUP, NT], bf16)  # [dv, head, tok]
            maskb = persist.tile([128, 4, TOK], bf16)
            bq_s = persist.tile([128, GROUP], f32)
            bk_s = persist.tile([128, 1], f32)
            bv_s = persist.tile([128, 1], f32)
            ones128 = persist.tile([128, 128], bf16)
            ident = persist.tile([128, 128], bf16)

            # ---------------- Phase 1: q/k/v projections ----------------
            with (
                tc.tile_pool(name="wproj", bufs=1) as wpool,
                tc.tile_pool(name="xtp", bufs=16) as xpool,
                tc.tile_pool(name="vtstage", bufs=1) as vtp,
                tc.psum_pool(name="ps1", bufs=4) as ps1,
                tc.psum_pool(name="pstr", bufs=2) as pstr,
            ):
                wk_s = wpool.tile([128, DC, 128], bf16)
                wv_s = wpool.tile([128, DC, 128], bf16)
                wq_s = wpool.tile([128, GROUP * DC, 128], bf16)

                # critical-path-first DMA ordering: the very first matmuls
                # need only wk chunk 0 + x chunk (0,0); v-proj of tile 0
                # needs wv shortly after; q-proj of tile 0 needs wq head 0
                # before the second x tile.
                nc.sync.dma_start(out=bk_s[:], in_=bk_d[:])
                nc.sync.dma_start(out=wk_s[:, 0:XC, :], in_=wk_d[:, 0:XC, :])
                xch = {}

                def xdma(tt, c):
                    xc = xpool.tile([128, XC, TOK], bf16, tag="xt")
                    xch[(tt, c)] = xc
                    nc.sync.dma_start(
                        out=xc[:], in_=xt_d[:, c * XC : (c + 1) * XC, ts(tt, TOK)]
                    )

                xdma(0, 0)
                for cc in range(1, NXC):
                    nc.sync.dma_start(
                        out=wk_s[:, cc * XC : (cc + 1) * XC, :],
                        in_=wk_d[:, cc * XC : (cc + 1) * XC, :],
                    )
                xdma(0, 1)
                nc.sync.dma_start(out=bv_s[:], in_=bv_d[:])
                nc.sync.dma_start(out=wv_s[:], in_=wv_d[:])
                xdma(0, 2)
                xdma(0, 3)
                nc.sync.dma_start(out=bq_s[:], in_=bq_d[:])
                nc.sync.dma_start(out=wq_s[:, ts(0, DC), :], in_=wq_d[:, ts(0, DC), :])
                for c in range(4, NXC):
                    xdma(0, c)
                for c in range(NXC):
                    xdma(1, c)
                for dq in range(1, GROUP):
                    nc.sync.dma_start(
                        out=wq_s[:, ts(dq, DC), :], in_=wq_d[:, ts(dq, DC), :]
                    )
                nc.sync.dma_start(out=maskb[:], in_=mask_d[:])
                make_identity(nc, ident[:])
                nc.vector.memset(ones128[:], 1.0)
                vT = vtp.tile([128, NT], bf16)

                for tt in range(NTT):
                    if tt + 2 < NTT:  # prefetch x two tiles ahead
                        for c in range(NXC):
                            xdma(tt + 2, c)

                    def xsrc(Dc):
                        return xch[(tt, Dc // XC)][:, Dc % XC, :]

                    ps = ps1.tile([128, TOK], f32)
                    for Dc in range(DC):
                        nc.tensor.matmul(
                            ps[:],
                            lhsT=wk_s[:, Dc, :],
                            rhs=xsrc(Dc),
                            start=(Dc == 0),
                            stop=(Dc == DC - 1),
                        )
                    nc.scalar.activation(
                        out=kT[:, ts(tt, TOK)],
                        in_=ps[:],
                        func=AF.Identity,
                        bias=bk_s[:, 0:1],
                    )
                    ps = ps1.tile([128, TOK], f32)
                    for Dc in range(DC):
                        nc.tensor.matmul(
                            ps[:],
                            lhsT=wv_s[:, Dc, :],
                            rhs=xsrc(Dc),
                            start=(Dc == 0),
                            stop=(Dc == DC - 1),
                        )
                    nc.scalar.activation(
                        out=vT[:, ts(tt, TOK)],
                        in_=ps[:],
                        func=AF.Identity,
                        bias=bv_s[:, 0:1],
                    )
                    for dq in range(GROUP):
                        ps = ps1.tile([128, TOK], f32)
                        for Dc in range(DC):
                            nc.tensor.matmul(
                                ps[:],
                                lhsT=wq_s[:, dq * DC + Dc, :],
                                rhs=xsrc(Dc),
                                start=(Dc == 0),
                                stop=(Dc == DC - 1),
                            )
                        nc.scalar.activation(
                            out=qT[:, dq, ts(tt, TOK)],
                            in_=ps[:],
                            func=AF.Identity,
                            bias=bq_s[:, dq : dq + 1],
                        )

                # transpose v to token-major tiles
                for t in range(NT // 128):
                    pt = pstr.tile([128, 128], bf16)
                    nc.tensor.transpose(pt[:], vT[:, ts(t, 128)], ident[:])
                    nc.vector.tensor_copy(out=vtm[:, t, :], in_=pt[:])

            # ---------------- Phase 2: causal attention ----------------
            with tc.tile_pool(name="wout", bufs=1) as wop:
                wo_s = wop.tile([128, GROUP * DC, 128], bf16)
                nc.sync.dma_start(out=wo_s[:], in_=wo_d[:])

                with (
                    tc.psum_pool(name="att_st", bufs=2) as stp,
                    tc.psum_pool(name="att_y", bufs=2) as yp,
                    tc.psum_pool(name="att_l", bufs=2) as lpool,
                    tc.tile_pool(name="ptile", bufs=6) as ppool,
                    tc.tile_pool(name="acc", bufs=3) as accp,
                    tc.tile_pool(name="invl", bufs=3) as invp,
                ):
                    pending_tail = [None]

                    def chain_tail(b, h, qi, yps, accEO):
                        # row sums via one ones-matmul pair, then 1/l scale.
                        # Deferred past the next chain's first pair so the PE
                        # isn't stalled on the DVE accumulate latency.
                        def tail():
                            lps = lpool.tile([128, TOK], f32)
                            nc.tensor.matmul(
                                lps[:],
                                lhsT=ones128[:],
                                rhs=accEO[:, 0, :],
                                start=True,
                                stop=False,
                            )
                            nc.tensor.matmul(
                                lps[:],
                                lhsT=ones128[:],
                                rhs=accEO[:, 1, :],
                                start=False,
                                stop=True,
                            )
                            inv = invp.tile([128, TOK], f32)
                            nc.vector.reciprocal_approx_fast(
                                out=inv[:], in_=lps[:]
                            )
                            nc.vector.tensor_mul(
                                out=yT[:, h, ds(b * T + qi * TOK, TOK)],
                                in0=yps[:],
                                in1=inv[:],
                            )

                        return tail

                    for bh in range(B * GROUP):
                        b, h = divmod(bh, GROUP)
                        for qi in range(T // TOK):
                            njt = 4 * (qi + 1)  # causal k-tiles of 128
                            yps = yp.tile([128, TOK], f32)
                            accEO = accp.tile([128, 2, TOK], bf16)
                            for jp in range(njt // 2):
                                st = stp.tile([128, 2, TOK], f32)
                                for jj in range(2):
                                    j = jp * 2 + jj
                                    r = j - 4 * qi
                                    diag = r >= 0
                                    nlo = max(r, 0) * 128
                                    nc.tensor.matmul(
                                        st[:, jj, nlo:TOK],
                                        lhsT=kT[:, ds(b * T + j * 128, 128)],
                                        rhs=qT[
                                            :,
                                            h,
                                            ds(b * T + qi * TOK + nlo, TOK - nlo),
                                        ],
                                        start=True,
                                        stop=not diag,
                                        skip_group_check=diag,
                                    )
                                    if diag:
                                        # left cols [0,nlo) have has_written
                                        # clear -> mask OVERWRITES them with
                                        # -1e9; [nlo,nlo+128) accumulates the
                                        # causal triangle onto the scores.
                                        nc.tensor.matmul(
                                            st[:, jj, 0 : nlo + 128],
                                            lhsT=ident[:],
                                            rhs=maskb[:, r, 0 : nlo + 128],
                                            start=False,
                                            stop=True,
                                            skip_group_check=True,
                                        )
                                ptile = ppool.tile([128, 2, TOK], bf16)
                                nc.scalar.activation(
                                    out=ptile[:], in_=st[:], func=AF.Exp
                                )
                                # running bf16 sum of P tiles (softmax denom)
                                if jp == 0:
                                    nc.vector.tensor_copy(
                                        out=accEO[:], in_=ptile[:]
                                    )
                                else:
                                    nc.vector.scalar_tensor_tensor(
                                        out=accEO[:],
                                        in0=ptile[:],
                                        scalar=1.0,
                                        in1=accEO[:],
                                        op0=ALU.mult,
                                        op1=ALU.add,
                                    )
                                for jj in range(2):
                                    j = jp * 2 + jj
                                    nlo = max(j - 4 * qi, 0) * 128
                                    nc.tensor.matmul(
                                        yps[:, nlo:TOK],
                                        lhsT=vtm[:, b * KT_PER_B + j, :],
                                        rhs=ptile[:, jj, nlo:TOK],
                                        start=(j == 0),
                                        stop=(j == njt - 1),
                                        skip_group_check=(
                                            nlo > 0 or j == njt - 1
                                        ),
                                    )
                                if jp == 0 and pending_tail[0] is not None:
                                    pending_tail[0]()
                                    pending_tail[0] = None
                            pending_tail[0] = chain_tail(b, h, qi, yps, accEO)
                    pending_tail[0]()

                # ---------------- Phase 3: partial out-projection ----------------
                with (
                    tc.psum_pool(name="ps3", bufs=6) as ps3,
                    tc.tile_pool(name="stg", bufs=6) as stg,
                ):
                    # tt outer: the last attention chain (tt=7) is only
                    # needed at the very end, hiding the phase transition.
                    for tt in range(NTT):
                        for Do in range(DC):
                            ps = ps3.tile([128, TOK], f32)
                            for c in range(GROUP):
                                nc.tensor.matmul(
                                    ps[:],
                                    lhsT=wo_s[:, c * DC + Do, :],
                                    rhs=yT[:, c, ts(tt, TOK)],
                                    start=(c == 0),
                                    stop=(c == GROUP - 1),
                                )
                            so = stg.tile([128, TOK], bf16)
                            # alternate copies between DVE and ACT so neither
                            # becomes the phase bottleneck
                            if Do % 2 == 0:
                                nc.vector.tensor_copy(out=so[:], in_=ps[:])
                            else:
                                nc.scalar.copy(out=so[:], in_=ps[:])
                            nc.sync.dma_start(
                                out=out_d[:, Do, ts(tt, TOK)], in_=so[:]
                            )

    if not nc.is_finalized():
        nc.finalize()
    return nc


def _prep_inputs(hidden_states, Wq, bq, Wk, bk, Wv, bv, Wo, bo):
    scale = 1.0 / math.sqrt(d)

    x_flat = np.asarray(hidden_states, dtype=np.float32).reshape(NT, D)
    # xt[p, Dc, t] = x[t, Dc*128+p]
    xt = np.ascontiguousarray(
        x_flat.reshape(NT, DC, 128).transpose(2, 1, 0)
    ).astype(BF16)

    jj = np.arange(128)[:, None, None]
    rr = np.arange(4)[None, :, None] * 128
    ii = np.arange(TOK)[None, None, :]
    masks = np.where(jj + rr > ii, np.float32(-1e9), np.float32(0.0)).astype(BF16)
    masks = np.ascontiguousarray(masks)

    in_maps = []
    for g in range(NC_):
        Wq_g = np.asarray(Wq[g * 512 : (g + 1) * 512, :], dtype=np.float32) * scale
        bq_g = np.asarray(bq[g * 512 : (g + 1) * 512], dtype=np.float32) * scale
        Wk_g = np.asarray(Wk[g * 128 : (g + 1) * 128, :], dtype=np.float32)
        bk_g = np.asarray(bk[g * 128 : (g + 1) * 128], dtype=np.float32)
        Wv_g = np.asarray(Wv[g * 128 : (g + 1) * 128], dtype=np.float32)
        bv_g = np.asarray(bv[g * 128 : (g + 1) * 128], dtype=np.float32)
        Wo_g = np.asarray(Wo[:, g * 512 : (g + 1) * 512], dtype=np.float32)

        # wq[p, dq*DC+Dc, m] = Wq_g[dq*128+m, Dc*128+p]
        wq_host = np.ascontiguousarray(
            Wq_g.reshape(GROUP, 128, DC, 128).transpose(3, 0, 2, 1).reshape(
                128, GROUP * DC, 128
            )
        ).astype(BF16)
        # wk[p, Dc, m] = Wk_g[m, Dc*128+p]
        wk_host = np.ascontiguousarray(
            Wk_g.reshape(128, DC, 128).transpose(2, 1, 0)
        ).astype(BF16)
        wv_host = np.ascontiguousarray(
            Wv_g.reshape(128, DC, 128).transpose(2, 1, 0)
        ).astype(BF16)
        # wo[p, c*DC+Do, m] = Wo_g[Do*128+m, c*128+p]
        wo_host = np.ascontiguousarray(
            Wo_g.reshape(DC, 128, GROUP, 128).transpose(3, 2, 0, 1).reshape(
                128, GROUP * DC, 128
            )
        ).astype(BF16)

        in_maps.append(
            {
                "xt": xt,
                "wq": wq_host,
                "wk": wk_host,
                "wv": wv_host,
                "wo": wo_host,
                "bq": np.ascontiguousarray(bq_g.reshape(GROUP, 128).T),
                "bk": bk_g.reshape(128, 1).copy(),
                "bv": bv_g.reshape(128, 1).copy(),
                "masks": masks,
            }
        )
    return in_maps


def kernel(
    hidden_states, Wq, bq, Wk, bk, Wv, bv, Wo, bo, _trace=False, _result_box=None
):
    from concourse.bass_utils import run_bass_kernel_spmd

    if "nc" not in _program_cache:
        _program_cache["nc"] = _build_program()
    nc = _program_cache["nc"]

    in_maps = _prep_inputs(hidden_states, Wq, bq, Wk, bk, Wv, bv, Wo, bo)
    res = run_bass_kernel_spmd(
        nc, in_maps, core_ids=list(range(NC_)), trace=_trace
    )
    if _result_box is not None:
        _result_box.append(res)

    acc = np.zeros((128, DC, NT), dtype=np.float32)
    for r in res.results:
        acc += np.asarray(r["out"], dtype=np.float32)
    # outT[Do*128+p, t] = acc[p, Do, t];  out[t, :] = outT[:, t] + bo
    outT = acc.transpose(1, 0, 2).reshape(D, NT)
    out = outT.T + np.asarray(bo, dtype=np.float32)[None, :]
    return np.ascontiguousarray(out.reshape(B, T, D), dtype=np.float32)
